# revision 39
# baseline (speedup 1.0000x reference)
"""Trainium2 Bass kernel for nn_BertDeAttention (dual cross-attention BERT block).

Strategy: data-parallel over batch (8 batches -> 8 NeuronCores). Each core runs
both attention branches for its batch:
  c_out = LN(attn(q=qin, kv=cin; Wq,Wk,Wv) @ Wo.T + bo + cin)
  q_out = LN(attn(q=cin, kv=qin; Wqq,Wqk,Wqv) @ Wo.T + bo + cin)

v6: fp8(e4m3)+DoubleRow matmuls for all four projections and the PV
(prob@V) contraction -- each MM contracts 256 elements (two 128-chunks via
the DoubleRow pair dim in the AP). Weights are host-prescaled by 64 (fp8
subnormal avoidance); the 1/64 is folded into the PSUM->SBUF evacuation op.
Scores stay bf16 (K=64 head pairs on disjoint PE row groups). ctx is scaled
by 16 into fp8; the out-projection evacuation divides by 64*16.

Device layouts:
  - activations enter feature-major fp8 xT [e, l]; weights as W.T*64 fp8
  - Q/K projections produce feature-major bf16 [o, l] head-pair tiles
  - V is token-major fp8 packed per head pair as [v_h0 | ones | v_h1] so a
    single MM per head yields both context rows and the softmax denominator
  - scores are computed transposed St[k, q]; attention mask rides the ACT
    per-partition bias slot of the fused exp(0.125*S + m); exp outputs fp8
  - softmax normalization (recip * 16) happens on the PSUM->SBUF pass of PV
  - out-projection consumes fp8 ctx; LayerNorm epilogue is token-major with
    bn_stats/bn_aggr + a DVE-only Newton rsqrt (keeps ACT exp table resident)
"""
import sys
import numpy as np

sys.path.insert(0, "/opt/trn_rl_repo")

import ml_dtypes  # noqa: E402

VERSION = "v10"
B, L, HID, NH = 8, 1024, 1024, 16
DH = HID // NH  # 64
NP = 128        # partitions
NCH = HID // NP  # 8 chunks of 128 along any 1024 dim
NCP = NCH // 2   # 4 chunk-pairs (DoubleRow contracts 2 chunks per MM)
NPAIR = NH // 2  # 8 head pairs
EPS = 1e-12
WSCALE = 64.0    # host premultiplier on weights before fp8 cast
CTXSCALE = 16.0  # ctx premultiplier into fp8 (undone in out-proj evac)

_COMPILED = {}


def _build(flags):
    import concourse.bass as bass  # noqa: F401
    import concourse.tile as tile
    from concourse import bacc, mybir

    BF16 = mybir.dt.bfloat16
    F32 = mybir.dt.float32
    F8 = mybir.dt.float8e4
    Alu = mybir.AluOpType
    Act = mybir.ActivationFunctionType
    DR = mybir.MatmulPerfMode.DoubleRow

    has_gb = flags["has_gb"]
    has_vb = flags.get("has_vb", False)
    reps = flags.get("reps", 1)

    nc = bacc.Bacc("TRN2", target_bir_lowering=False, debug=False)

    # ---- DRAM parameters -------------------------------------------------
    def din(name, shape, dt):
        return nc.dram_tensor(name, shape, dt, kind="ExternalInput").ap()

    xt_c = din("xt_c", [HID, L], F8)        # cin^T feature-major fp8
    xt_q = din("xt_q", [HID, L], F8)        # qin^T feature-major fp8
    cin32 = din("cin32", [L, HID], F32)     # residual (token-major, + bo)
    mask_c = din("mask_c", [NP, NCH], F32)  # mask[k] at [k%128, k//128]
    mask_q = din("mask_q", [NP, NCH], F32)
    wts = {n: din(f"wt_{n}", [HID, HID], F8)
           for n in ["q", "k", "v", "qq", "qk", "qv", "o"]}  # W.T * 64
    biases = {n: din(f"b_{n}", [NP, NCH], F32)
              for n in ["q", "k", "qq", "qk"]}               # [o%128, o//128]
    bvbc = {n: din(f"bvbc_{n}", [NP, NCH, NP], F32)
            for n in ["v", "qv"]}                            # bias_v bcast
    if has_gb:
        gb_in = din("gammabeta", [2, HID], F32)

    c_out = nc.dram_tensor("c_out", [L, HID], F32, kind="ExternalOutput").ap()
    q_out = nc.dram_tensor("q_out", [L, HID], F32, kind="ExternalOutput").ap()

    with tile.TileContext(nc) as tc:
        import contextlib
        ctx = contextlib.ExitStack()
        # SBUF pools
        xpool = ctx.enter_context(tc.tile_pool(name="x", bufs=1))
        vp = ctx.enter_context(tc.tile_pool(name="vp", bufs=9))
        qkp = ctx.enter_context(tc.tile_pool(name="qkp", bufs=8))
        esp = ctx.enter_context(tc.tile_pool(name="esp", bufs=2))
        ctxp = ctx.enter_context(tc.tile_pool(name="ctxp", bufs=2))
        rbcp = ctx.enter_context(tc.tile_pool(name="rbcp", bufs=1))
        epi = ctx.enter_context(tc.tile_pool(name="epi", bufs=2))
        cinp = ctx.enter_context(tc.tile_pool(name="cinp", bufs=2))
        smal = ctx.enter_context(tc.tile_pool(name="smal", bufs=4))
        # PSUM pools (8 banks: st 2x1 + pv 2x1 + proj 2x1)
        stp = ctx.enter_context(tc.tile_pool(name="stp", bufs=2, space="PSUM"))
        pvp = ctx.enter_context(tc.tile_pool(name="pvp", bufs=2, space="PSUM"))
        prp = ctx.enter_context(tc.tile_pool(name="prp", bufs=2, space="PSUM"))

        # ---- resident loads (outside the rep loop) -----------------------
        xc = xpool.tile([NP, NCH, L], F8, tag="xc")
        nc.sync.dma_start(xc[:], xt_c.rearrange("(c p) l -> p c l", p=NP))
        xq = xpool.tile([NP, NCH, L], F8, tag="xq")
        nc.sync.dma_start(xq[:], xt_q.rearrange("(c p) l -> p c l", p=NP))

        wsb = {}
        for n in ["q", "k", "v", "qq", "qk", "qv", "o"]:
            t = xpool.tile([NP, NCH, HID], F8, tag=f"w{n}")
            nc.gpsimd.dma_start(t[:], wts[n].rearrange("(c p) o -> p c o", p=NP))
            wsb[n] = t

        mset = {}
        for nm, src in [("c", mask_c), ("q", mask_q)]:
            m = smal.tile([NP, NCH], F32, tag=f"mask{nm}")
            nc.sync.dma_start(m[:], src[:])
            mset[nm] = m
        bset = {}
        for nm in ["q", "k", "qq", "qk"]:
            b = smal.tile([NP, NCH], F32, tag=f"b{nm}")
            nc.sync.dma_start(b[:], biases[nm][:])
            bset[nm] = b
        bvset = {}
        for nm in ["v", "qv"]:
            b = xpool.tile([NP, NCH, NP], F32, tag=f"bv{nm}")
            nc.sync.dma_start(b[:], bvbc[nm][:])
            bvset[nm] = b
        eps_sb = smal.tile([NP, 1], F32, tag="eps")
        nc.vector.memset(eps_sb[:], EPS)
        if has_gb:
            gb_bc = xpool.tile([NP, 2, HID], F32, tag="gb")
            import concourse.bass as _b
            gb_src = _b.AP(tensor=gb_in.tensor, offset=gb_in.offset,
                           ap=[[0, NP]] + list(gb_in.ap))
            nc.gpsimd.dma_start(gb_bc[:], gb_src)

        IWS = 1.0 / WSCALE
        IOS = 1.0 / (WSCALE * CTXSCALE)

        # ================= per-branch program ============================
        loop_cm = tc.For_i(0, reps, 1) if reps > 1 else contextlib.nullcontext()
        ctx.enter_context(loop_cm)

        def projections(br):
            """Allocate this branch's V/Q/K tiles and return (tiles, closures).
            closures emit one projection group each; the caller interleaves
            them into the other branch's attention stream."""
            xsrc_q, xsrc_kv = (xq, xc) if br == "c" else (xc, xq)
            wn_q, wn_k, wn_v = ("q", "k", "v") if br == "c" else ("qq", "qk", "qv")
            wv = wsb[wn_v]
            bv = bvset[wn_v if wn_v == "v" else "qv"]
            vgrp = [vp.tile([NP, 4, NCH, 192], F8, tag="vp", bufs=4,
                            name=f"vg_{br}{oh}") for oh in range(2)]
            qt = [qkp.tile([NP, L], BF16, tag="qt", name=f"qt_{br}{p}")
                  for p in range(NPAIR)]
            kt = [qkp.tile([NP, L], BF16, tag="kt", name=f"kt_{br}{p}")
                  for p in range(NPAIR)]

            # ---- V projection: token-major fp8 [l, o] packed [v_h0|1|v_h1]
            def v_group(lc):
                if lc == 0:
                    for oh in range(2):
                        for a in range(4):
                            nc.vector.memset(vgrp[oh][:, a, :, 64:128], 1.0)
                for oh in range(2):
                    ps = prp.tile([NP, 512], F32, tag="pr")
                    # second group accumulates in reversed chunk order so the
                    # group-boundary Ldweights pair is identical -> deduped
                    cs = range(NCP) if oh == 0 else range(NCP - 1, -1, -1)
                    for j, c in enumerate(cs):
                        nc.tensor.matmul(
                            ps[:],
                            xsrc_kv[:, 2 * c:2 * c + 2, lc * NP:(lc + 1) * NP],
                            wv[:, 2 * c:2 * c + 2, oh * 512:(oh + 1) * 512],
                            start=(j == 0), stop=(j == NCP - 1), perf_mode=DR)
                    src = ps[:].rearrange("p (a h d) -> p a h d", a=4, h=2)
                    dst = vgrp[oh][:, :, lc, :].rearrange(
                        "p a (g d) -> p a g d", d=64)  # [p, 4, 3, 64]
                    bsl = bv[:, 4 * oh:4 * oh + 4, :].rearrange(
                        "p a (h d) -> p a h d", h=2)
                    # dst groups 0/2 -> v_h0 (cols 0:64), v_h1 (128:192);
                    # one 3D-output op per head (walrus rejects 4D outs)
                    for h in range(2):
                        nc.vector.scalar_tensor_tensor(
                            out=dst[:, :, 2 * h, :], in0=src[:, :, h, :],
                            scalar=IWS, op0=Alu.mult,
                            in1=bsl[:, :, h, :], op1=Alu.add)

            # ---- Q/K projections: feature-major bf16 pair tiles [128, L]
            def qk_group(wn, t, p, xsrc):
                w = wsb[wn]
                bias = bset[wn]
                for lh in range(2):
                    ps = prp.tile([NP, 512], F32, tag="pr")
                    cs = range(NCP) if lh == 0 else range(NCP - 1, -1, -1)
                    for j, c in enumerate(cs):
                        nc.tensor.matmul(
                            ps[:],
                            w[:, 2 * c:2 * c + 2, p * NP:(p + 1) * NP],
                            xsrc[:, 2 * c:2 * c + 2, lh * 512:(lh + 1) * 512],
                            start=(j == 0), stop=(j == NCP - 1), perf_mode=DR)
                    nc.vector.tensor_scalar(
                        out=t[:, lh * 512:(lh + 1) * 512], in0=ps[:],
                        scalar1=IWS, scalar2=bias[:, p:p + 1],
                        op0=Alu.mult, op1=Alu.add)

            vcl = [(lambda lc=lc: v_group(lc)) for lc in range(NCH)]
            qkcl = []
            for p in range(NPAIR):
                qkcl.append(lambda p=p: qk_group(wn_q, qt[p], p, xsrc_q))
                qkcl.append(lambda p=p: qk_group(wn_k, kt[p], p, xsrc_kv))
            vtiles = [vgrp[p // 4][:, p % 4] for p in range(NPAIR)]
            return (vtiles, qt, kt), vcl, qkcl

        def attention(br, vtiles, qt, kt, do_pv=True, side=None):
            # Software-pipelined: scores+exp for iteration i+1 are emitted
            # BEFORE the PV matmuls of iteration i, so the strict-FIFO PE
            # queue never stalls the ACT exp stream behind a PV matmul that
            # waits on an exp.
            msk = mset["c" if br == "c" else "q"]
            cx = ctxp.tile([NP, NPAIR, L], F8, tag="ctx")

            def scores(p):
                # both qh halves of a head share one kt stationary slice ->
                # the second Ldweights dedupes away
                es = esp.tile([NP, NCH, 2, 2, 512], F8, tag="es")
                for kc in range(NCH):
                    ksl = slice(kc * NP, (kc + 1) * NP)
                    for h in range(2):
                        hsl = slice(64 * h, 64 * h + 64)
                        st = stp.tile([NP, 2, 512], F32, tag="st")
                        for qh in range(2):
                            nc.tensor.matmul(
                                st[:, qh, :], kt[p][hsl, ksl],
                                qt[p][hsl, qh * 512:(qh + 1) * 512],
                                start=True, stop=True)
                        nc.scalar.activation(
                            es[:, kc, h, :, :].rearrange("p a b -> p (a b)"),
                            st[:].rearrange("p a b -> p (a b)"),
                            Act.Exp, bias=msk[:, kc:kc + 1], scale=0.125)
                return es

            def pv_norm(p, es):
                for qh in range(2):
                    qsl = slice(qh * 512, (qh + 1) * 512)
                    pv0 = pvp.tile([NP, 512], F32, tag="pv")
                    pv1 = pvp.tile([NP, 512], F32, tag="pv")
                    for c in range(NCP):
                        nc.tensor.matmul(
                            pv0[:], vtiles[p][:, 2 * c:2 * c + 2, 0:128],
                            es[:, 2 * c:2 * c + 2, 0, qh, :],
                            start=(c == 0), stop=(c == NCP - 1), perf_mode=DR)
                        nc.tensor.matmul(
                            pv1[:], vtiles[p][:, 2 * c:2 * c + 2, 64:192],
                            es[:, 2 * c:2 * c + 2, 1, qh, :],
                            start=(c == 0), stop=(c == NCP - 1), perf_mode=DR)
                    # softmax-normalize into fp8 ctx (scaled by 16)
                    # pv0: rows 0:64 = ctx_h0, 64:128 = rowsum (bcast)
                    # pv1: rows 0:64 = rowsum (bcast), 64:128 = ctx_h1
                    rbc = rbcp.tile([NP, 512], F32, tag="rbc")
                    nc.vector.reciprocal(rbc[0:64, :], pv0[64:128, :])
                    nc.vector.scalar_tensor_tensor(
                        out=cx[0:64, p, qsl], in0=pv0[0:64, :],
                        scalar=CTXSCALE, op0=Alu.mult,
                        in1=rbc[0:64, :], op1=Alu.mult)
                    nc.vector.reciprocal(rbc[64:128, :], pv1[0:64, :])
                    nc.vector.scalar_tensor_tensor(
                        out=cx[64:128, p, qsl], in0=pv1[64:128, :],
                        scalar=CTXSCALE, op0=Alu.mult,
                        in1=rbc[64:128, :], op1=Alu.mult)

            prev = None
            for p in range(NPAIR):
                es = scores(p)
                if prev is not None and do_pv:
                    pv_norm(*prev)
                prev = (p, es)
                if side is not None and p < len(side):
                    for fn in side[p]:
                        fn()
            if do_pv:
                pv_norm(*prev)
            return cx

        def out_proj(br, cx, out_dram):
            return [(lambda lc=lc: out_group(br, cx, out_dram, lc))
                    for lc in range(NCH)]

        def out_group(br, cx, out_dram, lc):
            wo = wsb["o"]
            if True:
                cint = cinp.tile([NP, HID], F32, tag="cin")
                nc.sync.dma_start(cint[:], cin32[lc * NP:(lc + 1) * NP, :])
                y = epi.tile([NP, HID], F32, tag="y")
                for oh in range(2):
                    ps = prp.tile([NP, 512], F32, tag="pr")
                    cs = range(NCP) if oh == 0 else range(NCP - 1, -1, -1)
                    for j, c in enumerate(cs):
                        nc.tensor.matmul(
                            ps[:],
                            cx[:, 2 * c:2 * c + 2, lc * NP:(lc + 1) * NP],
                            wo[:, 2 * c:2 * c + 2, oh * 512:(oh + 1) * 512],
                            start=(j == 0), stop=(j == NCP - 1), perf_mode=DR)
                    nc.vector.scalar_tensor_tensor(
                        out=y[:, oh * 512:(oh + 1) * 512], in0=ps[:],
                        scalar=IOS, op0=Alu.mult,
                        in1=cint[:, oh * 512:(oh + 1) * 512],
                        op1=Alu.add)
                stats = smal.tile([NP, 2, 6], F32, tag="stats")
                for oh in range(2):
                    nc.vector.bn_stats(stats[:, oh, :],
                                       y[:, oh * 512:(oh + 1) * 512])
                mv = smal.tile([NP, 2], F32, tag="mv")
                nc.vector.bn_aggr(mv[:], stats[:])
                # rstd = rsqrt(var + eps) via DVE-only Newton iteration
                # (keeps ACT exp table resident; x0 = min(1, 1/v) converges)
                w = smal.tile([NP, 3], F32, tag="nwt")
                v_ = w[:, 0:1]
                x_ = w[:, 1:2]
                u_ = w[:, 2:3]
                nc.vector.tensor_scalar(out=v_, in0=mv[:, 1:2],
                                        scalar1=eps_sb[:], scalar2=None,
                                        op0=Alu.add)
                nc.vector.reciprocal(x_, v_)
                nc.vector.tensor_scalar(out=x_, in0=x_, scalar1=1.0,
                                        scalar2=None, op0=Alu.min)
                for _ in range(3):
                    nc.vector.tensor_tensor(out=u_, in0=x_, in1=x_,
                                            op=Alu.mult)
                    nc.vector.tensor_tensor(out=u_, in0=u_, in1=v_,
                                            op=Alu.mult)
                    nc.vector.tensor_scalar(out=u_, in0=u_, scalar1=-0.5,
                                            scalar2=1.5, op0=Alu.mult,
                                            op1=Alu.add)
                    nc.vector.tensor_tensor(out=x_, in0=x_, in1=u_,
                                            op=Alu.mult)
                nc.vector.tensor_scalar(
                    out=y[:], in0=y[:], scalar1=mv[:, 0:1],
                    scalar2=x_, op0=Alu.subtract, op1=Alu.mult)
                if has_gb:
                    nc.vector.tensor_tensor(
                        out=y[:], in0=y[:], in1=gb_bc[:, 0, :], op=Alu.mult)
                    nc.vector.tensor_tensor(
                        out=y[:], in0=y[:], in1=gb_bc[:, 1, :], op=Alu.add)
                nc.gpsimd.dma_start(out_dram[lc * NP:(lc + 1) * NP, :], y[:])

        def emit_plain(vcl, qkcl):
            for fn in vcl:
                fn()
            for fn in qkcl:
                fn()

        def sched_proj(vcl, qkcl):
            """One V group + the pair's Q/K groups per attention iteration;
            qt/kt ring-slot writes land right after the consuming branch's
            pair-p scores finish, so they never stall any queue."""
            return [[vcl[p], qkcl[2 * p], qkcl[2 * p + 1]]
                    for p in range(NPAIR)]

        # emission: c projections plain; q projections interleaved into the
        # ACT-bound c attention; c out-proj groups interleaved into q
        # attention; q out-proj at the rep seam.
        only = flags.get("only")
        c_tiles, c_vcl, c_qkcl = projections("c")
        emit_plain(c_vcl, c_qkcl)
        if only == "proj":
            q_tiles, q_vcl, q_qkcl = projections("q")
            emit_plain(q_vcl, q_qkcl)
        elif only in ("exp", "attn"):
            attention("c", *c_tiles, do_pv=(only == "attn"))
            q_tiles, q_vcl, q_qkcl = projections("q")
            emit_plain(q_vcl, q_qkcl)
            attention("q", *q_tiles, do_pv=(only == "attn"))
        else:
            q_tiles, q_vcl, q_qkcl = projections("q")
            c_cx = attention("c", *c_tiles, side=sched_proj(q_vcl, q_qkcl))
            ocl = out_proj("c", c_cx, c_out)
            q_cx = attention("q", *q_tiles,
                             side=[[ocl[p]] for p in range(NPAIR)])
            for fn in out_proj("q", q_cx, q_out):
                fn()
        ctx.close()
    n_dedup = _dedupe_ldw(nc, mybir)
    assert n_dedup > 0 or flags.get("only")
    nc.compile()
    return nc


def _dedupe_ldw(nc, mybir):
    """Remove a Ldweights whose weights AP is identical to the immediately
    preceding Ldweights on the PE stream (post-scheduling order): the PE
    array still holds those weights, so the following non-self-loading
    Matmult can reuse them. The removed instruction's sync deps are merged
    into its Matmult and dangling dep references are remapped to it."""
    name_map = {}
    for f in nc.m.functions:
        for b in f.blocks:
            insts = list(b.instructions)
            out, last_sig, pending = [], None, None
            for inst in insts:
                if str(inst.engine).endswith("PE"):
                    if isinstance(inst, mybir.InstLdweights):
                        pap = inst.ins[0]
                        ba = pap.bass_ap
                        sig = (ba.tensor.name, int(ba.offset),
                               tuple(tuple(x) for x in ba.ap),
                               str(pap.dtype), str(inst.perf_mode),
                               str(inst.tile_position),
                               str(inst.is_transpose))
                        if sig == last_sig and pending is None:
                            pending = inst
                            continue
                        last_sig = sig
                    else:
                        if pending is not None:
                            inst.add_sync_dependencies_from(
                                pending.sync_dependency_set_copy())
                            name_map[pending.name] = inst.name
                            pending = None
                out.append(inst)
            assert pending is None, "dangling deduped ldweights"
            b.instructions = out
    if name_map:
        for f in nc.m.functions:
            for b in f.blocks:
                for inst in b.instructions:
                    inst.remap_dependency_names(name_map)
    return len(name_map)


def _prep(inputs):
    f8 = ml_dtypes.float8_e4m3

    def t_f8(a):
        at = np.ascontiguousarray(np.asarray(a, np.float32).T)
        return np.clip(at, -240.0, 240.0).astype(f8)

    def w_f8(a):
        at = np.ascontiguousarray(np.asarray(a, np.float32).T) * WSCALE
        return np.clip(at, -240.0, 240.0).astype(f8)

    wts = {}
    for n, key in [("q", "Wq"), ("k", "Wk"), ("v", "Wv"), ("qq", "Wqq"),
                   ("qk", "Wqk"), ("qv", "Wqv"), ("o", "Wo")]:
        wts[n] = w_f8(inputs[key])

    def b_rs(b):
        return np.ascontiguousarray(
            np.asarray(b, np.float32).reshape(NCH, NP).T)

    shared = {f"wt_{n}": w for n, w in wts.items()}
    for n, key in [("q", "bq"), ("k", "bk"), ("qq", "bqq"), ("qk", "bqk")]:
        shared[f"b_{n}"] = b_rs(inputs[key])
    for n, key in [("v", "bv"), ("qv", "bqv")]:
        bb = np.asarray(inputs[key], np.float32)
        shared[f"bvbc_{n}"] = np.ascontiguousarray(
            np.broadcast_to(bb, (NP, HID)).reshape(NP, NCH, NP))
    gamma = np.asarray(inputs["gamma"], np.float32)
    beta = np.asarray(inputs["beta"], np.float32)
    has_gb = not (np.all(gamma == 1.0) and np.all(beta == 0.0))
    has_vb = bool(np.any(np.asarray(inputs["bv"], np.float32)) or
                  np.any(np.asarray(inputs["bqv"], np.float32)))
    if has_gb:
        shared["gammabeta"] = np.ascontiguousarray(
            np.stack([gamma, beta], 0))

    cin = np.asarray(inputs["cinput_tensor"], np.float32)
    qin = np.asarray(inputs["qinput_tensor"], np.float32)
    bo = np.asarray(inputs["bo"], np.float32)  # folded into the residual
    am = np.asarray(inputs["attention_mask"], np.float32).reshape(B, L)
    qam = np.asarray(inputs["qattention_mask"], np.float32).reshape(B, L)

    in_maps = []
    for b in range(B):
        m = dict(shared)
        m["xt_c"] = t_f8(cin[b])
        m["xt_q"] = t_f8(qin[b])
        m["cin32"] = np.ascontiguousarray(cin[b] + bo)
        m["mask_c"] = np.ascontiguousarray(am[b].reshape(NCH, NP).T)
        m["mask_q"] = np.ascontiguousarray(qam[b].reshape(NCH, NP).T)
        in_maps.append(m)
    return in_maps, has_gb, has_vb


def kernel(**inputs):
    from concourse.bass_utils import run_bass_kernel_spmd

    in_maps, has_gb, has_vb = _prep(inputs)
    key = (VERSION, has_gb, has_vb)
    if key not in _COMPILED:
        _COMPILED[key] = _build({"has_gb": has_gb, "has_vb": has_vb})
    nc = _COMPILED[key]
    res = run_bass_kernel_spmd(nc, in_maps, list(range(B)))
    c = np.stack([res.results[b]["c_out"] for b in range(B)], 0)
    q = np.stack([res.results[b]["q_out"] for b in range(B)], 0)
    return (c, q)


# revision 41
# speedup vs baseline: 1.1467x; 1.1467x over previous
"""Trainium2 Bass kernel for nn_BertDeAttention (dual cross-attention BERT block).

Strategy: data-parallel over batch (8 batches -> 8 NeuronCores). Each core runs
both attention branches for its batch:
  c_out = LN(attn(q=qin, kv=cin; Wq,Wk,Wv) @ Wo.T + bo + cin)
  q_out = LN(attn(q=cin, kv=qin; Wqq,Wqk,Wqv) @ Wo.T + bo + cin)

v6: fp8(e4m3)+DoubleRow matmuls for all four projections and the PV
(prob@V) contraction -- each MM contracts 256 elements (two 128-chunks via
the DoubleRow pair dim in the AP). Weights are host-prescaled by 64 (fp8
subnormal avoidance); the 1/64 is folded into the PSUM->SBUF evacuation op.
Scores stay bf16 (K=64 head pairs on disjoint PE row groups). ctx is scaled
by 16 into fp8; the out-projection evacuation divides by 64*16.

Device layouts:
  - activations enter feature-major fp8 xT [e, l]; weights as W.T*64 fp8
  - Q/K projections produce feature-major bf16 [o, l] head-pair tiles
  - V is token-major fp8 packed per head pair as [v_h0 | ones | v_h1] so a
    single MM per head yields both context rows and the softmax denominator
  - scores are computed transposed St[k, q]; attention mask rides the ACT
    per-partition bias slot of the fused exp(0.125*S + m); exp outputs fp8
  - softmax normalization (recip * 16) happens on the PSUM->SBUF pass of PV
  - out-projection consumes fp8 ctx; LayerNorm epilogue is token-major with
    bn_stats/bn_aggr + a DVE-only Newton rsqrt (keeps ACT exp table resident)
"""
import sys
import numpy as np

sys.path.insert(0, "/opt/trn_rl_repo")

import ml_dtypes  # noqa: E402

VERSION = "v11"
B, L, HID, NH = 8, 1024, 1024, 16
DH = HID // NH  # 64
NP = 128        # partitions
NCH = HID // NP  # 8 chunks of 128 along any 1024 dim
NCP = NCH // 2   # 4 chunk-pairs (DoubleRow contracts 2 chunks per MM)
NPAIR = NH // 2  # 8 head pairs
EPS = 1e-12
WSCALE = 64.0    # host premultiplier on weights before fp8 cast
CTXSCALE = 16.0  # ctx premultiplier into fp8 (undone in out-proj evac)

_COMPILED = {}


def _build(flags):
    import concourse.bass as bass  # noqa: F401
    import concourse.tile as tile
    from concourse import bacc, mybir

    BF16 = mybir.dt.bfloat16
    F32 = mybir.dt.float32
    F8 = mybir.dt.float8e4
    Alu = mybir.AluOpType
    Act = mybir.ActivationFunctionType
    DR = mybir.MatmulPerfMode.DoubleRow

    has_gb = flags["has_gb"]
    has_vb = flags.get("has_vb", False)
    reps = flags.get("reps", 1)

    nc = bacc.Bacc("TRN2", target_bir_lowering=False, debug=False)

    # ---- DRAM parameters -------------------------------------------------
    def din(name, shape, dt):
        return nc.dram_tensor(name, shape, dt, kind="ExternalInput").ap()

    xt_c = din("xt_c", [HID, L], F8)        # cin^T feature-major fp8
    xt_q = din("xt_q", [HID, L], F8)        # qin^T feature-major fp8
    cin32 = din("cin32", [L, HID], F32)     # residual (token-major, + bo)
    mask_c = din("mask_c", [NP, NCH], F32)  # mask[k] at [k%128, k//128]
    mask_q = din("mask_q", [NP, NCH], F32)
    wts = {n: din(f"wt_{n}", [HID, HID], F8)
           for n in ["q", "k", "v", "qq", "qk", "qv", "o"]}  # W.T * 64
    biases = {n: din(f"b_{n}", [NP, NCH], F32)
              for n in ["q", "k", "qq", "qk"]}               # [o%128, o//128]
    bvbc = {n: din(f"bvbc_{n}", [NP, NCH, NP], F32)
            for n in ["v", "qv"]}                            # bias_v bcast
    if has_gb:
        gb_in = din("gammabeta", [2, HID], F32)

    c_out = nc.dram_tensor("c_out", [L, HID], F32, kind="ExternalOutput").ap()
    q_out = nc.dram_tensor("q_out", [L, HID], F32, kind="ExternalOutput").ap()

    with tile.TileContext(nc) as tc:
        import contextlib
        ctx = contextlib.ExitStack()
        # SBUF pools
        xpool = ctx.enter_context(tc.tile_pool(name="x", bufs=1))
        vp = ctx.enter_context(tc.tile_pool(name="vp", bufs=9))
        qkp = ctx.enter_context(tc.tile_pool(name="qkp", bufs=8))
        esp = ctx.enter_context(tc.tile_pool(name="esp", bufs=2))
        ctxp = ctx.enter_context(tc.tile_pool(name="ctxp", bufs=2))
        rbcp = ctx.enter_context(tc.tile_pool(name="rbcp", bufs=1))
        epi = ctx.enter_context(tc.tile_pool(name="epi", bufs=2))
        cinp = ctx.enter_context(tc.tile_pool(name="cinp", bufs=2))
        smal = ctx.enter_context(tc.tile_pool(name="smal", bufs=4))
        # PSUM pools (8 banks: st 2x1 + pv 2x1 + proj 2x1)
        stp = ctx.enter_context(tc.tile_pool(name="stp", bufs=2, space="PSUM"))
        pvp = ctx.enter_context(tc.tile_pool(name="pvp", bufs=2, space="PSUM"))
        prp = ctx.enter_context(tc.tile_pool(name="prp", bufs=2, space="PSUM"))

        # ---- resident loads (outside the rep loop) -----------------------
        xc = xpool.tile([NP, NCH, L], F8, tag="xc")
        nc.sync.dma_start(xc[:], xt_c.rearrange("(c p) l -> p c l", p=NP))
        xq = xpool.tile([NP, NCH, L], F8, tag="xq")
        nc.sync.dma_start(xq[:], xt_q.rearrange("(c p) l -> p c l", p=NP))

        wsb = {}
        for n in ["q", "k", "v", "qq", "qk", "qv", "o"]:
            t = xpool.tile([NP, NCH, HID], F8, tag=f"w{n}")
            nc.gpsimd.dma_start(t[:], wts[n].rearrange("(c p) o -> p c o", p=NP))
            wsb[n] = t

        mset = {}
        for nm, src in [("c", mask_c), ("q", mask_q)]:
            m = smal.tile([NP, NCH], F32, tag=f"mask{nm}")
            nc.sync.dma_start(m[:], src[:])
            mset[nm] = m
        bset = {}
        for nm in ["q", "k", "qq", "qk"]:
            b = smal.tile([NP, NCH], F32, tag=f"b{nm}")
            nc.sync.dma_start(b[:], biases[nm][:])
            bset[nm] = b
        bvset = {}
        for nm in ["v", "qv"]:
            b = xpool.tile([NP, NCH, NP], F32, tag=f"bv{nm}")
            nc.sync.dma_start(b[:], bvbc[nm][:])
            bvset[nm] = b
        eps_sb = smal.tile([NP, 1], F32, tag="eps")
        nc.vector.memset(eps_sb[:], EPS)
        if has_gb:
            gb_bc = xpool.tile([NP, 2, HID], F32, tag="gb")
            import concourse.bass as _b
            gb_src = _b.AP(tensor=gb_in.tensor, offset=gb_in.offset,
                           ap=[[0, NP]] + list(gb_in.ap))
            nc.gpsimd.dma_start(gb_bc[:], gb_src)

        IWS = 1.0 / WSCALE
        IOS = 1.0 / (WSCALE * CTXSCALE)

        # ================= per-branch program ============================
        loop_cm = tc.For_i(0, reps, 1) if reps > 1 else contextlib.nullcontext()
        ctx.enter_context(loop_cm)

        def projections(br):
            """Allocate this branch's V/Q/K tiles and return (tiles, closures).
            closures emit one projection group each; the caller interleaves
            them into the other branch's attention stream."""
            xsrc_q, xsrc_kv = (xq, xc) if br == "c" else (xc, xq)
            wn_q, wn_k, wn_v = ("q", "k", "v") if br == "c" else ("qq", "qk", "qv")
            wv = wsb[wn_v]
            bv = bvset[wn_v if wn_v == "v" else "qv"]
            vgrp = [vp.tile([NP, 4, NCH, 192], F8, tag="vp", bufs=4,
                            name=f"vg_{br}{oh}") for oh in range(2)]
            qt = [qkp.tile([NP, L], BF16, tag="qt", name=f"qt_{br}{p}")
                  for p in range(NPAIR)]
            kt = [qkp.tile([NP, L], BF16, tag="kt", name=f"kt_{br}{p}")
                  for p in range(NPAIR)]

            # ---- V projection: token-major fp8 [l, o] packed [v_h0|1|v_h1]
            def v_group(lc):
                if lc == 0:
                    for oh in range(2):
                        for a in range(4):
                            nc.vector.memset(vgrp[oh][:, a, :, 64:128], 1.0)
                for oh in range(2):
                    ps = prp.tile([NP, 512], F32, tag="pr")
                    # second group accumulates in reversed chunk order so the
                    # group-boundary Ldweights pair is identical -> deduped
                    cs = range(NCP) if oh == 0 else range(NCP - 1, -1, -1)
                    for j, c in enumerate(cs):
                        nc.tensor.matmul(
                            ps[:],
                            xsrc_kv[:, 2 * c:2 * c + 2, lc * NP:(lc + 1) * NP],
                            wv[:, 2 * c:2 * c + 2, oh * 512:(oh + 1) * 512],
                            start=(j == 0), stop=(j == NCP - 1), perf_mode=DR)
                    src = ps[:].rearrange("p (a h d) -> p a h d", a=4, h=2)
                    dst = vgrp[oh][:, :, lc, :].rearrange(
                        "p a (g d) -> p a g d", d=64)  # [p, 4, 3, 64]
                    bsl = bv[:, 4 * oh:4 * oh + 4, :].rearrange(
                        "p a (h d) -> p a h d", h=2)
                    # dst groups 0/2 -> v_h0 (cols 0:64), v_h1 (128:192);
                    # one 3D-output op per head (walrus rejects 4D outs)
                    for h in range(2):
                        nc.vector.scalar_tensor_tensor(
                            out=dst[:, :, 2 * h, :], in0=src[:, :, h, :],
                            scalar=IWS, op0=Alu.mult,
                            in1=bsl[:, :, h, :], op1=Alu.add)

            # ---- Q/K projections: feature-major bf16 pair tiles [128, L]
            def qk_group(wn, t, p, xsrc):
                w = wsb[wn]
                bias = bset[wn]
                for lh in range(2):
                    ps = prp.tile([NP, 512], F32, tag="pr")
                    cs = range(NCP) if lh == 0 else range(NCP - 1, -1, -1)
                    for j, c in enumerate(cs):
                        nc.tensor.matmul(
                            ps[:],
                            w[:, 2 * c:2 * c + 2, p * NP:(p + 1) * NP],
                            xsrc[:, 2 * c:2 * c + 2, lh * 512:(lh + 1) * 512],
                            start=(j == 0), stop=(j == NCP - 1), perf_mode=DR)
                    nc.vector.tensor_scalar(
                        out=t[:, lh * 512:(lh + 1) * 512], in0=ps[:],
                        scalar1=IWS, scalar2=bias[:, p:p + 1],
                        op0=Alu.mult, op1=Alu.add)

            vcl = [(lambda lc=lc: v_group(lc)) for lc in range(NCH)]
            qkcl = []
            for p in range(NPAIR):
                qkcl.append(lambda p=p: qk_group(wn_q, qt[p], p, xsrc_q))
                qkcl.append(lambda p=p: qk_group(wn_k, kt[p], p, xsrc_kv))
            vtiles = [vgrp[p // 4][:, p % 4] for p in range(NPAIR)]
            return (vtiles, qt, kt), vcl, qkcl

        def attention(br, vtiles, qt, kt, do_pv=True, side=None):
            # Software-pipelined: scores+exp for iteration i+1 are emitted
            # BEFORE the PV matmuls of iteration i, so the strict-FIFO PE
            # queue never stalls the ACT exp stream behind a PV matmul that
            # waits on an exp.
            msk = mset["c" if br == "c" else "q"]
            cx = ctxp.tile([NP, NPAIR, L], F8, tag="ctx")

            def scores(p, qh):
                qsl = slice(qh * 512, (qh + 1) * 512)
                es = esp.tile([NP, NCH, 2, 512], F8, tag="es", bufs=3)
                for kc in range(NCH):
                    st = stp.tile([NP, 2, 512], F32, tag="st")
                    nc.tensor.matmul(
                        st[:, 0, :],
                        kt[p][0:64, kc * NP:(kc + 1) * NP],
                        qt[p][0:64, qsl], start=True, stop=True)
                    nc.tensor.matmul(
                        st[:, 1, :],
                        kt[p][64:128, kc * NP:(kc + 1) * NP],
                        qt[p][64:128, qsl], start=True, stop=True)
                    nc.scalar.activation(
                        es[:, kc, :, :].rearrange("p a b -> p (a b)"),
                        st[:].rearrange("p a b -> p (a b)"),
                        Act.Exp, bias=msk[:, kc:kc + 1], scale=0.125)
                return es

            def pv_norm(p, qh, es):
                qsl = slice(qh * 512, (qh + 1) * 512)
                pv0 = pvp.tile([NP, 512], F32, tag="pv")
                pv1 = pvp.tile([NP, 512], F32, tag="pv")
                for c in range(NCP):
                    nc.tensor.matmul(
                        pv0[:], vtiles[p][:, 2 * c:2 * c + 2, 0:128],
                        es[:, 2 * c:2 * c + 2, 0, :],
                        start=(c == 0), stop=(c == NCP - 1), perf_mode=DR)
                    nc.tensor.matmul(
                        pv1[:], vtiles[p][:, 2 * c:2 * c + 2, 64:192],
                        es[:, 2 * c:2 * c + 2, 1, :],
                        start=(c == 0), stop=(c == NCP - 1), perf_mode=DR)
                # softmax-normalize into fp8 ctx (scaled by 16)
                # pv0: rows 0:64 = ctx_h0, 64:128 = rowsum (bcast)
                # pv1: rows 0:64 = rowsum (bcast), 64:128 = ctx_h1
                rbc = rbcp.tile([NP, 512], F32, tag="rbc")
                nc.vector.reciprocal(rbc[0:64, :], pv0[64:128, :])
                nc.vector.scalar_tensor_tensor(
                    out=cx[0:64, p, qsl], in0=pv0[0:64, :],
                    scalar=CTXSCALE, op0=Alu.mult,
                    in1=rbc[0:64, :], op1=Alu.mult)
                nc.vector.reciprocal(rbc[64:128, :], pv1[0:64, :])
                nc.vector.scalar_tensor_tensor(
                    out=cx[64:128, p, qsl], in0=pv1[64:128, :],
                    scalar=CTXSCALE, op0=Alu.mult,
                    in1=rbc[64:128, :], op1=Alu.mult)

            prev = None
            it = 0
            for p in range(NPAIR):
                for qh in range(2):
                    es = scores(p, qh)
                    if prev is not None and do_pv:
                        pv_norm(*prev)
                    prev = (p, qh, es)
                    if side is not None and it < len(side):
                        for fn in side[it]:
                            fn()
                    it += 1
            if do_pv:
                pv_norm(*prev)
            return cx

        def out_proj(br, cx, out_dram):
            return [(lambda lc=lc: out_group(br, cx, out_dram, lc))
                    for lc in range(NCH)]

        def out_group(br, cx, out_dram, lc):
            wo = wsb["o"]
            if True:
                cint = cinp.tile([NP, HID], F32, tag="cin")
                nc.sync.dma_start(cint[:], cin32[lc * NP:(lc + 1) * NP, :])
                y = epi.tile([NP, HID], F32, tag="y")
                for oh in range(2):
                    ps = prp.tile([NP, 512], F32, tag="pr")
                    cs = range(NCP) if oh == 0 else range(NCP - 1, -1, -1)
                    for j, c in enumerate(cs):
                        nc.tensor.matmul(
                            ps[:],
                            cx[:, 2 * c:2 * c + 2, lc * NP:(lc + 1) * NP],
                            wo[:, 2 * c:2 * c + 2, oh * 512:(oh + 1) * 512],
                            start=(j == 0), stop=(j == NCP - 1), perf_mode=DR)
                    nc.vector.scalar_tensor_tensor(
                        out=y[:, oh * 512:(oh + 1) * 512], in0=ps[:],
                        scalar=IOS, op0=Alu.mult,
                        in1=cint[:, oh * 512:(oh + 1) * 512],
                        op1=Alu.add)
                stats = smal.tile([NP, 2, 6], F32, tag="stats")
                for oh in range(2):
                    nc.vector.bn_stats(stats[:, oh, :],
                                       y[:, oh * 512:(oh + 1) * 512])
                mv = smal.tile([NP, 2], F32, tag="mv")
                nc.vector.bn_aggr(mv[:], stats[:])
                # rstd = rsqrt(var + eps) via DVE-only Newton iteration
                # (keeps ACT exp table resident; x0 = min(1, 1/v) converges)
                w = smal.tile([NP, 3], F32, tag="nwt")
                v_ = w[:, 0:1]
                x_ = w[:, 1:2]
                u_ = w[:, 2:3]
                nc.vector.tensor_scalar(out=v_, in0=mv[:, 1:2],
                                        scalar1=eps_sb[:], scalar2=None,
                                        op0=Alu.add)
                nc.vector.reciprocal(x_, v_)
                nc.vector.tensor_scalar(out=x_, in0=x_, scalar1=1.0,
                                        scalar2=None, op0=Alu.min)
                for _ in range(3):
                    nc.vector.tensor_tensor(out=u_, in0=x_, in1=x_,
                                            op=Alu.mult)
                    nc.vector.tensor_tensor(out=u_, in0=u_, in1=v_,
                                            op=Alu.mult)
                    nc.vector.tensor_scalar(out=u_, in0=u_, scalar1=-0.5,
                                            scalar2=1.5, op0=Alu.mult,
                                            op1=Alu.add)
                    nc.vector.tensor_tensor(out=x_, in0=x_, in1=u_,
                                            op=Alu.mult)
                nc.vector.tensor_scalar(
                    out=y[:], in0=y[:], scalar1=mv[:, 0:1],
                    scalar2=x_, op0=Alu.subtract, op1=Alu.mult)
                if has_gb:
                    nc.vector.tensor_tensor(
                        out=y[:], in0=y[:], in1=gb_bc[:, 0, :], op=Alu.mult)
                    nc.vector.tensor_tensor(
                        out=y[:], in0=y[:], in1=gb_bc[:, 1, :], op=Alu.add)
                nc.gpsimd.dma_start(out_dram[lc * NP:(lc + 1) * NP, :], y[:])

        def emit_plain(vcl, qkcl):
            for fn in vcl:
                fn()
            for fn in qkcl:
                fn()

        def sched_proj(vcl, qkcl):
            """Interleave V groups on even iterations, Q/K pair-groups right
            after the consuming branch's pair finishes (iteration 2p+1) so
            the qt/kt ring-slot writes never stall any queue."""
            side = []
            for i in range(16):
                fns = []
                if i % 2 == 0:
                    fns.append(vcl[i // 2])
                else:
                    p = (i - 1) // 2
                    fns += [qkcl[2 * p], qkcl[2 * p + 1]]
                side.append(fns)
            return side

        # emission: c projections plain; q projections interleaved into the
        # ACT-bound c attention; c out-proj groups interleaved into q
        # attention; q out-proj at the rep seam.
        only = flags.get("only")
        c_tiles, c_vcl, c_qkcl = projections("c")
        emit_plain(c_vcl, c_qkcl)
        if only == "proj":
            q_tiles, q_vcl, q_qkcl = projections("q")
            emit_plain(q_vcl, q_qkcl)
        elif only in ("exp", "attn"):
            attention("c", *c_tiles, do_pv=(only == "attn"))
            q_tiles, q_vcl, q_qkcl = projections("q")
            emit_plain(q_vcl, q_qkcl)
            attention("q", *q_tiles, do_pv=(only == "attn"))
        else:
            q_tiles, q_vcl, q_qkcl = projections("q")
            c_cx = attention("c", *c_tiles, side=sched_proj(q_vcl, q_qkcl))
            ocl = out_proj("c", c_cx, c_out)
            q_cx = attention("q", *q_tiles,
                             side=[[ocl[i // 2]] if i % 2 == 0 else []
                                   for i in range(16)])
            for fn in out_proj("q", q_cx, q_out):
                fn()
        ctx.close()
    n_dedup = _dedupe_ldw(nc, mybir)
    assert n_dedup > 0 or flags.get("only")
    nc.compile()
    return nc


def _dedupe_ldw(nc, mybir):
    """Remove a Ldweights whose weights AP is identical to the immediately
    preceding Ldweights on the PE stream (post-scheduling order): the PE
    array still holds those weights, so the following non-self-loading
    Matmult can reuse them. The removed instruction's sync deps are merged
    into its Matmult and dangling dep references are remapped to it."""
    name_map = {}
    for f in nc.m.functions:
        for b in f.blocks:
            insts = list(b.instructions)
            out, last_sig, pending = [], None, None
            for inst in insts:
                if str(inst.engine).endswith("PE"):
                    if isinstance(inst, mybir.InstLdweights):
                        pap = inst.ins[0]
                        ba = pap.bass_ap
                        sig = (ba.tensor.name, int(ba.offset),
                               tuple(tuple(x) for x in ba.ap),
                               str(pap.dtype), str(inst.perf_mode),
                               str(inst.tile_position),
                               str(inst.is_transpose))
                        if sig == last_sig and pending is None:
                            pending = inst
                            continue
                        last_sig = sig
                    else:
                        if pending is not None:
                            inst.add_sync_dependencies_from(
                                pending.sync_dependency_set_copy())
                            name_map[pending.name] = inst.name
                            pending = None
                out.append(inst)
            assert pending is None, "dangling deduped ldweights"
            b.instructions = out
    if name_map:
        for f in nc.m.functions:
            for b in f.blocks:
                for inst in b.instructions:
                    inst.remap_dependency_names(name_map)
    return len(name_map)


def _prep(inputs):
    f8 = ml_dtypes.float8_e4m3

    def t_f8(a):
        at = np.ascontiguousarray(np.asarray(a, np.float32).T)
        return np.clip(at, -240.0, 240.0).astype(f8)

    def w_f8(a):
        at = np.ascontiguousarray(np.asarray(a, np.float32).T) * WSCALE
        return np.clip(at, -240.0, 240.0).astype(f8)

    wts = {}
    for n, key in [("q", "Wq"), ("k", "Wk"), ("v", "Wv"), ("qq", "Wqq"),
                   ("qk", "Wqk"), ("qv", "Wqv"), ("o", "Wo")]:
        wts[n] = w_f8(inputs[key])

    def b_rs(b):
        return np.ascontiguousarray(
            np.asarray(b, np.float32).reshape(NCH, NP).T)

    shared = {f"wt_{n}": w for n, w in wts.items()}
    for n, key in [("q", "bq"), ("k", "bk"), ("qq", "bqq"), ("qk", "bqk")]:
        shared[f"b_{n}"] = b_rs(inputs[key])
    for n, key in [("v", "bv"), ("qv", "bqv")]:
        bb = np.asarray(inputs[key], np.float32)
        shared[f"bvbc_{n}"] = np.ascontiguousarray(
            np.broadcast_to(bb, (NP, HID)).reshape(NP, NCH, NP))
    gamma = np.asarray(inputs["gamma"], np.float32)
    beta = np.asarray(inputs["beta"], np.float32)
    has_gb = not (np.all(gamma == 1.0) and np.all(beta == 0.0))
    has_vb = bool(np.any(np.asarray(inputs["bv"], np.float32)) or
                  np.any(np.asarray(inputs["bqv"], np.float32)))
    if has_gb:
        shared["gammabeta"] = np.ascontiguousarray(
            np.stack([gamma, beta], 0))

    cin = np.asarray(inputs["cinput_tensor"], np.float32)
    qin = np.asarray(inputs["qinput_tensor"], np.float32)
    bo = np.asarray(inputs["bo"], np.float32)  # folded into the residual
    am = np.asarray(inputs["attention_mask"], np.float32).reshape(B, L)
    qam = np.asarray(inputs["qattention_mask"], np.float32).reshape(B, L)

    in_maps = []
    for b in range(B):
        m = dict(shared)
        m["xt_c"] = t_f8(cin[b])
        m["xt_q"] = t_f8(qin[b])
        m["cin32"] = np.ascontiguousarray(cin[b] + bo)
        m["mask_c"] = np.ascontiguousarray(am[b].reshape(NCH, NP).T)
        m["mask_q"] = np.ascontiguousarray(qam[b].reshape(NCH, NP).T)
        in_maps.append(m)
    return in_maps, has_gb, has_vb


def kernel(**inputs):
    from concourse.bass_utils import run_bass_kernel_spmd

    in_maps, has_gb, has_vb = _prep(inputs)
    key = (VERSION, has_gb, has_vb)
    if key not in _COMPILED:
        _COMPILED[key] = _build({"has_gb": has_gb, "has_vb": has_vb})
    nc = _COMPILED[key]
    res = run_bass_kernel_spmd(nc, in_maps, list(range(B)))
    c = np.stack([res.results[b]["c_out"] for b in range(B)], 0)
    q = np.stack([res.results[b]["q_out"] for b in range(B)], 0)
    return (c, q)
